# revision 38
# baseline (speedup 1.0000x reference)
"""AttentionDecoder single-step kernel for 8 TRN2 NeuronCores.

Math (see reference):
    e      = emb[token]                                   (E,)
    s      = [e,h] @ W_attn.T + b_attn                    (L,)
    a      = softmax(s)                                   (L,)
    ctx    = a @ enc                                      (H,)
    x      = relu([e,ctx] @ W_comb.T + b_comb)            (H,)
    gates  = x @ W_ih.T + h @ W_hh.T + b_ih + b_hh        (4H,)
    c'     = sig(f)*c + sig(i)*tanh(g)
    h'     = sig(o)*tanh(c')
    logp   = log_softmax(h' @ W_out.T + b_out)            (V,)

Everything is a batch-1 matvec, so the kernel is HBM-bandwidth bound on the
~833 MB of weights.  Sharding (8 cores):
    W_attn  row (output L) sharded     -> AllGather of the 256 scores/core
    enc     column (H) sharded         -> ctx slice per core
    W_comb  input (contraction) shard  -> AllReduce of partial x
    W_ih/hh row (gate dim) sharded     -> local c'/h' slices
    W_out   input (h) sharded          -> AllReduce of partial logits z

Device layout convention: every length-N vector lives in SBUF as
[128, N/128] with element (p, c) = v[c*128 + p] ("chunk-per-column"), so it
is directly usable as matmul lhsT/rhs chunks and all pointwise ops run
128-lanes wide.  Weights are pre-transposed and pre-tiled on the host into
one flat [128, tiles*128] blob per stage, streamed through an SBUF ring by
the sync engine while the PE consumes [128,128] stationary tiles.
"""

import os
from dataclasses import dataclass, field

import numpy as np


# ---------------------------------------------------------------------------
# configuration
# ---------------------------------------------------------------------------

@dataclass(frozen=True)
class Cfg:
    H: int = 4096          # hidden = embedding = vocab
    L: int = 2048          # encoder length
    NC: int = 8            # cores
    RING: int = 16         # weight ring slots
    TPC: int = 32          # tiles (128x128) per DMA chunk
    wdt: str = "bf16"      # matmul operand dtype: "bf16" or "f32"
    warm: bool = False     # warmup collective deadlocks ncfw; keep off
    rdma: bool = True      # hand-rolled collectives via remote SBUF DMA

    @property
    def EH(self):
        return 2 * self.H

    @property
    def L_sh(self):
        return self.L // self.NC

    @property
    def H_sh(self):
        return self.H // self.NC

    # per-stage (contraction chunks C, output tiles T)
    @property
    def Ca(self):
        return self.EH // 128

    @property
    def Ta(self):
        return self.L_sh // 128

    @property
    def Ce(self):
        return self.L // 128

    @property
    def Te(self):
        return self.H_sh // 128

    @property
    def Cc(self):
        return (2 * self.H_sh) // 128

    @property
    def Tc(self):
        return self.H // 128

    @property
    def Cg(self):
        return self.H // 128

    @property
    def Tg(self):
        return (4 * self.H_sh) // 128

    @property
    def Q(self):
        return self.Tg // 4

    @property
    def Co(self):
        return self.H_sh // 128

    @property
    def Tv(self):
        return self.H // 128

    @property
    def Tla(self):
        return self.NC * self.Ta  # attn prob columns (full L)

    # outbuf column offsets
    @property
    def OB_H(self):
        return self.Tv

    @property
    def OB_C(self):
        return self.Tv + self.Q

    @property
    def OB_A(self):
        return self.Tv + 2 * self.Q

    @property
    def OBW(self):
        return self.Tv + 2 * self.Q + self.Tla

    @property
    def CG(self):
        return self.TPC * 128  # ring slot columns


CFG = Cfg()


# stage table: (name, C, T, rhs source)  -- order == stream order
def _stages(c: Cfg):
    return [
        ("w_attn", c.Ca, c.Ta),
        ("w_enc", c.Ce, c.Te),
        ("w_comb", c.Cc, c.Tc),
        ("w_hh", c.Cg, c.Tg),
        ("w_ih", c.Cg, c.Tg),
        ("w_out", c.Co, c.Tv),
    ]


# ---------------------------------------------------------------------------
# host-side input prep
# ---------------------------------------------------------------------------

def _np_wdt(c):
    if c.wdt == "bf16":
        import ml_dtypes
        return ml_dtypes.bfloat16
    return np.float32


def _pack(arr_km: np.ndarray, C: int, T: int, dtype=np.float32) -> np.ndarray:
    """[C*128, T*128] lhsT matrix -> [128, C*T*128] tile stream.

    Tile order is (t outer, c inner) -- K-contiguous so each PSUM column's
    accumulation group closes before the next one starts (one open group per
    PSUM bank).  tile (t,c)[p,m] = arr[c*128+p, t*128+m].
    """
    assert arr_km.shape == (C * 128, T * 128), (arr_km.shape, C, T)
    return np.ascontiguousarray(
        arr_km.reshape(C, 128, T, 128).transpose(1, 2, 0, 3).reshape(128, C * T * 128),
        dtype=dtype,
    )


# Logical->physical NC map on TRN2 (driver BASE table).  The recursive-
# doubling exchange pairs cores by XOR on *physical* ids, so after the
# 3 rounds, score block j on core k holds the scores of logical core
# k ^ _GPERM[j].  The per-device constant XOR cancels out of the pairing.
_GPERM = (0, 1, 2, 3, 6, 7, 4, 5)


def _cols(v: np.ndarray, dtype=np.float32) -> np.ndarray:
    """length-N vector -> [128, N/128] chunk-per-column layout."""
    n = v.shape[0]
    assert n % 128 == 0
    return np.ascontiguousarray(v.reshape(n // 128, 128).T, dtype=dtype)


def prep_inputs(c: Cfg, inputs: dict) -> list[dict]:
    H, L, NC = c.H, c.L, c.NC
    token = np.asarray(inputs["token"])
    h0 = np.asarray(inputs["h"], np.float32)[0]
    c0 = np.asarray(inputs["c"], np.float32)[0]
    enc = np.asarray(inputs["encoder_outputs"], np.float32)
    emb = np.asarray(inputs["emb"], np.float32)
    W_attn = np.asarray(inputs["W_attn"], np.float32)
    b_attn = np.asarray(inputs["b_attn"], np.float32)
    W_comb = np.asarray(inputs["W_comb"], np.float32)
    b_comb = np.asarray(inputs["b_comb"], np.float32)
    W_ih = np.asarray(inputs["W_ih"], np.float32)
    W_hh = np.asarray(inputs["W_hh"], np.float32)
    b_ih = np.asarray(inputs["b_ih"], np.float32)
    b_hh = np.asarray(inputs["b_hh"], np.float32)
    W_out = np.asarray(inputs["W_out"], np.float32)
    b_out = np.asarray(inputs["b_out"], np.float32)

    e = emb[int(token.reshape(-1)[0])]
    eh = np.concatenate([e, h0])
    b_g = b_ih + b_hh

    W_attn_T = np.ascontiguousarray(W_attn.T)        # [2H, L]
    W_comb_T = np.ascontiguousarray(W_comb.T)        # [2H, H]
    W_ih_T = np.ascontiguousarray(W_ih.T)            # [H, 4H]
    W_hh_T = np.ascontiguousarray(W_hh.T)            # [H, 4H]
    W_out_T = np.ascontiguousarray(W_out.T)          # [H, V]

    in_maps = []
    for k in range(NC):
        ls = slice(k * c.L_sh, (k + 1) * c.L_sh)
        hs = slice(k * c.H_sh, (k + 1) * c.H_sh)
        # gate-dim columns of W_*_T for this core: (gate, u, p) order
        gcols = np.concatenate(
            [np.arange(g * H + k * c.H_sh, g * H + (k + 1) * c.H_sh) for g in range(4)]
        )
        wt = _np_wdt(c)
        m = {
            "pack_bf": np.concatenate(
                [_cols(eh, wt), _cols(h0, wt), _cols(e[hs], wt)], axis=1),
            "pack_f32": np.concatenate(
                [_cols(c0[hs]), _cols(b_attn[ls]), _cols(b_comb),
                 _cols(b_g[gcols]), _cols(b_out),
                 np.ones((128, 1), np.float32)], axis=1),
            "ones_row": np.ones((1, 128), np.float32),
            "w_attn": _pack(W_attn_T[:, ls], c.Ca, c.Ta, wt),
            "w_enc": _pack(
                np.concatenate(
                    [enc[(k ^ _GPERM[j]) * c.L_sh:((k ^ _GPERM[j]) + 1) * c.L_sh,
                         hs] for j in range(NC)]) if c.rdma else enc[:, hs],
                c.Ce, c.Te, wt),
            "w_comb": _pack(
                np.concatenate([W_comb_T[hs, :], W_comb_T[H + k * c.H_sh:
                                                          H + (k + 1) * c.H_sh, :]]),
                c.Cc, c.Tc, wt),
            "w_hh": _pack(W_hh_T[:, gcols], c.Cg, c.Tg, wt),
            "w_ih": _pack(W_ih_T[:, gcols], c.Cg, c.Tg, wt),
            "w_out": _pack(W_out_T[hs, :], c.Co, c.Tv, wt),
        }
        in_maps.append(m)
    return in_maps


def decode_outputs(c: Cfg, outs: list[np.ndarray]):
    """per-core [128, OBW] f32 -> (logp, h_new, c_new, attn_w) full tensors."""
    o0 = np.asarray(outs[0], np.float32)
    logp = o0[:, 0:c.Tv].T.reshape(-1)
    attn = o0[:, c.OB_A:c.OB_A + c.Tla].T.reshape(-1)
    if c.rdma:
        # core 0's score block j holds logical core _GPERM[j]'s slice
        blocks = attn.reshape(c.NC, c.L_sh)
        attn = np.concatenate([blocks[_GPERM[q]] for q in range(c.NC)])
    h_new = np.concatenate(
        [np.asarray(outs[k], np.float32)[:, c.OB_H:c.OB_H + c.Q].T.reshape(-1)
         for k in range(c.NC)]
    )
    c_new = np.concatenate(
        [np.asarray(outs[k], np.float32)[:, c.OB_C:c.OB_C + c.Q].T.reshape(-1)
         for k in range(c.NC)]
    )
    return (logp[None, :], h_new[None, :], c_new[None, :], attn[None, :])


# ---------------------------------------------------------------------------
# device graph
# ---------------------------------------------------------------------------

def build_nc(c: Cfg):
    import concourse.bass as bass
    import concourse.mybir as mybir
    from contextlib import ExitStack

    f32 = mybir.dt.float32
    wdt = mybir.dt.bfloat16 if c.wdt == "bf16" else mybir.dt.float32
    nc = bass.Bass(target_bir_lowering=False)
    core_ids = list(range(c.NC))

    # ---- DRAM parameters ------------------------------------------------
    # packed small params: one bf16 pack, one f32 pack, plus ones_row
    PBF = c.Ca + c.Cg + c.Cc // 2                 # eh | h | e_comb
    PF32 = c.Q + c.Ta + c.Tc + c.Tg + c.Tv + 1    # c | ba | bc | bg | bo | ones
    small = {
        "pack_bf": [128, PBF], "pack_f32": [128, PF32], "ones_row": [1, 128],
    }
    dparams = {}
    dparams["pack_bf"] = nc.declare_dram_parameter("pack_bf", [128, PBF], wdt,
                                                   isOutput=False)
    dparams["pack_f32"] = nc.declare_dram_parameter("pack_f32", [128, PF32], f32,
                                                    isOutput=False)
    dparams["ones_row"] = nc.declare_dram_parameter("ones_row", [1, 128], f32,
                                                    isOutput=False)
    for name, C, T in _stages(c):
        dparams[name] = nc.declare_dram_parameter(
            name, [128, C * T * 128], wdt, isOutput=False)
    out_ext = nc.declare_dram_parameter("out", [128, c.OBW], f32, isOutput=True)

    # collective bounce buffers (internal DRAM; outs must be Shared)
    warm_in = nc.dram_tensor("warm_in", [8], f32)
    warm_out = nc.dram_tensor("warm_out", [8 * c.NC], f32, addr_space="Shared")
    ag_in = nc.dram_tensor("ag_in", [c.L_sh], f32)
    ag_out = nc.dram_tensor("ag_out", [c.NC, 128, c.Ta], f32, addr_space="Shared")
    ar1_in = nc.dram_tensor("ar1_in", [128, c.Tc], f32)
    ar1_out = nc.dram_tensor("ar1_out", [128, c.Tc], f32, addr_space="Shared")
    ar2_in = nc.dram_tensor("ar2_in", [128, c.Tv], f32)
    ar2_out = nc.dram_tensor("ar2_out", [128, c.Tv], f32, addr_space="Shared")

    # chunk plan: (param_name, col_start, col_end) in stream order
    chunks = []
    F = {}  # stage name -> cumulative chunk count when stage fully consumed
    for name, C, T in _stages(c):
        total = C * T
        g = 0
        while g < total:
            g2 = min(g + c.TPC, total)
            chunks.append((name, g * 128, g2 * 128))
            g += c.TPC
        F[name] = len(chunks)

    ctx = ExitStack()
    with ctx:
        HALF = (c.RING + 1) // 2
        sb = {}
        for name, (shape, dt_) in {
            "wring0": ([128, HALF * c.CG], wdt),
            "wring1": ([128, (c.RING - HALF) * c.CG], wdt),
            "pbf_sb": ([128, c.Ca + c.Cg + c.Cc], wdt),
            "pf32_sb": ([128, PF32], f32),
            "x_sb": ([128, c.Cg], wdt),
            "xb_sb": ([128, c.Cg], f32),
            "s_sb": ([128, c.Ta], f32), "s_all": ([128, c.Tla], f32),
            "sacc": ([128, c.Tla], f32),
            "rx0": ([128, c.Tc], f32), "rx1": ([128, c.Tc], f32),
            "rx2": ([128, c.Tc], f32),
            "rz0": ([128, c.Tv], f32), "rz1": ([128, c.Tv], f32),
            "rz2": ([128, c.Tv], f32),
            "exp_sb": ([128, c.Tla], wdt), "exp32_sb": ([128, c.Tla], f32),
            "sumexp": ([128, 1], f32), "recip": ([128, 1], f32),
            "rs_sb": ([128, 1], f32),
            "ar1_sb": ([128, c.Tc], f32), "x_raw": ([128, c.Tc], f32),
            "g_sb": ([128, c.Tg], f32), "g1_sb": ([128, c.Tg], f32),
            "pw_sb": ([128, c.Tg], f32),
            "tc_sb": ([128, c.Q], f32), "t1_sb": ([128, c.Q], f32),
            "t2_sb": ([128, c.Q], f32), "hnew_bf": ([128, c.Q], wdt),
            "ar2_sb": ([128, c.Tv], f32), "z_raw": ([128, c.Tv], f32),
            "z_sb": ([128, c.Tv], f32),
            "ez_sb": ([128, c.Tv], f32), "sez": ([128, 1], f32),
            "lse_sb": ([128, 1], f32), "lbc_sb": ([128, 1], f32),
            "ones_r": ([1, 128], f32),
            "outbuf": ([128, c.OBW], f32),
        }.items():
            sb[name] = ctx.enter_context(nc.sbuf_tensor(name, shape, dt_))

        # views into the packed tensors (plain AP slices)
        pb, pf = sb["pbf_sb"], sb["pf32_sb"]
        o = [0]

        def _take(pack, n):
            a, o[0] = o[0], o[0] + n
            return pack[:, a:a + n]

        eh_v = _take(pb, c.Ca)
        h_v = _take(pb, c.Cg)
        comb_v = pb[:, o[0]:o[0] + c.Cc]   # e_comb cols + ctx cols
        o[0] = 0
        c_v = _take(pf, c.Q)
        battn_v = _take(pf, c.Ta)
        bcomb_v = _take(pf, c.Tc)
        bgate_v = _take(pf, c.Tg)
        bout_v = _take(pf, c.Tv)
        onesc_v = _take(pf, 1)

        ps = {}
        for name, shape in {
            "p_attn": [128, c.Ta], "p_enc": [128, c.Te], "p_comb": [128, c.Tc],
            "p_gh": [128, c.Tg], "p_gx": [128, c.Tg], "p_z": [128, c.Tv],
            "p_S": [128, 1], "p_bc": [128, 1],
        }.items():
            ps[name] = ctx.enter_context(nc.psum_tensor(name, shape, f32))

        sem = {}
        for name in ["vsem", "fsem", "pe", "dve", "act", "gc", "odsem", "g_warm",
                     "g_ag_in", "g_ag_out", "g_ar1_in", "g_ar1_out",
                     "g_ar2_in", "g_ar2_out",
                     "psem", "lsem", "gdone", "gadd",
                     "rs_s0", "rs_s1", "rs_s2", "rs_x0", "rs_x1", "rs_x2",
                     "rs_z0", "rs_z1", "rs_z2"]:
            sem[name] = ctx.enter_context(nc.semaphore(name))
        dsl = [ctx.enter_context(nc.semaphore(f"dsl{i}")) for i in range(c.RING)]

        # milestone counts
        N_SMALL = len(small)
        VS_ALL = 16 * N_SMALL
        PE_SUMS, PE_BC, PE_SUMZ, PE_LSEBC = range(1, 5)
        (A1_EXP, A2_EXP32, A3_RELU, A4_PW, A5_TANHC, A6_EZ,
         A7_LN) = range(1, 8)
        D1_S, D2_RECIP, D3_CTX, D4_AR1, D5_XB, D6_GB, D7_CNEW, D8_HNEW, \
            D9_AR2, D10_ZB, D11_LOGP = range(1, 12)

        small_src = {  # sbuf destination of each small param
            "pack_bf": pb[:, 0:PBF], "pack_f32": pf[:, :],
            "ones_row": sb["ones_r"][:, :],
        }
        OFF_EH, OFF_H, OFF_CB = 0, c.Ca, c.Ca + c.Cg

        with nc.Block() as block:

            DM = {}  # dve milestone name -> count
            dve_n = [0]

            def dinc(inst, name=None):
                dve_n[0] += 1
                inst.then_inc(sem["dve"], 1)
                if name:
                    DM[name] = dve_n[0]

            # ---------------- vector (DVE) -----------------------------------
            # emitted first so other engines can reference DM[...] counts
            @block.vector
            def _(v):
                Q = c.Q
                v.wait_ge(sem["fsem"], F["w_attn"])
                s_dst = sb["sacc"][:, 0:c.Ta] if c.rdma else sb["s_sb"][:, :]
                dinc(v.tensor_add(s_dst, ps["p_attn"][:, :], battn_v), "s")
                v.wait_ge(sem["pe"], PE_SUMS)
                dinc(v.reciprocal(sb["recip"][0:1, 0:1],
                                  ps["p_S"][0:1, 0:1]), "recip")
                v.wait_ge(sem["fsem"], F["w_enc"])
                v.wait_ge(sem["pe"], PE_BC)
                dinc(v.tensor_copy(sb["rs_sb"][:, :], ps["p_bc"][:, :]), "rs")
                v.wait_ge(sem["dve"], DM["rs"])
                dinc(v.tensor_scalar_mul(
                    pb[:, OFF_CB + c.Cc // 2:OFF_CB + c.Cc], ps["p_enc"][:, :],
                    sb["rs_sb"][:, 0:1]), "ctx")
                v.wait_ge(sem["act"], A2_EXP32)
                v.tensor_scalar_mul(
                    sb["outbuf"][:, c.OB_A:c.OB_A + c.Tla], sb["exp32_sb"][:, :],
                    sb["rs_sb"][:, 0:1])
                v.wait_ge(sem["fsem"], F["w_comb"])
                dinc(v.tensor_copy(sb["ar1_sb"][:, :], ps["p_comb"][:, :]), "ar1")
                if c.rdma:
                    v.wait_ge(sem["gdone"], 2)
                    x_src = sb["ar1_sb"][:, :]
                else:
                    v.wait_ge(sem["g_ar1_out"], 16)
                    x_src = sb["x_raw"][:, :]
                dinc(v.tensor_add(sb["xb_sb"][:, :], x_src, bcomb_v), "xb")
                v.wait_ge(sem["fsem"], F["w_ih"])
                dinc(v.tensor_add(sb["g1_sb"][:, :], ps["p_gh"][:, :],
                                  bgate_v), "g1")
                v.wait_ge(sem["dve"], DM["g1"])
                dinc(v.tensor_add(sb["g_sb"][:, :], sb["g1_sb"][:, :],
                                  ps["p_gx"][:, :]), "gb")
                v.wait_ge(sem["act"], A4_PW)
                v.tensor_mul(sb["t1_sb"][:, :], sb["pw_sb"][:, Q:2 * Q], c_v)
                dinc(v.tensor_mul(sb["t2_sb"][:, :], sb["pw_sb"][:, 0:Q],
                                  sb["pw_sb"][:, 2 * Q:3 * Q]), "t2")
                v.wait_ge(sem["dve"], DM["t2"])
                dinc(v.tensor_add(sb["outbuf"][:, c.OB_C:c.OB_C + Q],
                                  sb["t1_sb"][:, :], sb["t2_sb"][:, :]), "cnew")
                v.wait_ge(sem["act"], A5_TANHC)
                dinc(v.tensor_mul(sb["outbuf"][:, c.OB_H:c.OB_H + Q],
                                  sb["pw_sb"][:, 3 * Q:4 * Q],
                                  sb["tc_sb"][:, :]), "hnew")
                v.wait_ge(sem["dve"], DM["hnew"])
                dinc(v.tensor_copy(sb["hnew_bf"][:, :],
                                   sb["outbuf"][:, c.OB_H:c.OB_H + Q]), "hnewbf")
                v.wait_ge(sem["fsem"], F["w_out"])
                dinc(v.tensor_copy(sb["ar2_sb"][:, :], ps["p_z"][:, :]), "ar2")
                if c.rdma:
                    v.wait_ge(sem["gdone"], 3)
                    z_src = sb["ar2_sb"][:, :]
                else:
                    v.wait_ge(sem["g_ar2_out"], 16)
                    z_src = sb["z_raw"][:, :]
                dinc(v.tensor_add(sb["z_sb"][:, :], z_src, bout_v), "zb")
                v.wait_ge(sem["pe"], PE_LSEBC)
                dinc(v.tensor_copy(sb["lbc_sb"][:, :], ps["p_bc"][:, :]), "lbc")
                v.wait_ge(sem["dve"], DM["lbc"])
                dinc(v.tensor_scalar(
                    out=sb["outbuf"][:, 0:c.Tv], in0=sb["z_sb"][:, :],
                    scalar1=sb["lbc_sb"][:, 0:1], scalar2=None,
                    op0=mybir.AluOpType.subtract), "logp")

            # ---------------- sync: all the big streaming DMA ----------------
            @block.sync
            def _(sync):
                for name in small:
                    sync.dma_start(
                        out=small_src[name], in_=dparams[name][:, :]
                    ).then_inc(sem["vsem"], 16)
                for i, (name, c0_, c1_) in enumerate(chunks):
                    if i >= c.RING:
                        sync.wait_ge(sem["fsem"], i - c.RING + 1)
                    slot = i % c.RING
                    wr = sb["wring0"] if slot < HALF else sb["wring1"]
                    off = (slot if slot < HALF else slot - HALF) * c.CG
                    sync.dma_start(
                        out=wr[:, off:off + (c1_ - c0_)],
                        in_=dparams[name][:, c0_:c1_],
                    ).then_inc(dsl[slot], 16)
                sync.wait_ge(sem["dve"], DM["logp"])
                sync.dma_start(out=out_ext[:, :], in_=sb["outbuf"][:, :]).then_inc(
                    sem["odsem"], 16)
                sync.wait_ge(sem["odsem"], 16)

            # ---------------- tensor: every matmul ---------------------------
            @block.tensor
            def _(tensor):
                chunk_idx = [0]  # mutable closure counter

                def stage(psum, C, T, rhs_t, rhs_base=0, start_grp=True,
                          stop_grp=True, pre_wait=None):
                    if pre_wait is not None:
                        tensor.wait_ge(*pre_wait)
                    base = chunk_idx[0]
                    n_tiles = C * T
                    for g in range(n_tiles):
                        ti, ci = divmod(g, C)
                        local_chunk, off = divmod(g, c.TPC)
                        gl_chunk = base + local_chunk
                        if off == 0:
                            tensor.wait_ge(dsl[gl_chunk % c.RING],
                                           16 * (gl_chunk // c.RING + 1))
                        slot = gl_chunk % c.RING
                        wr = sb["wring0"] if slot < HALF else sb["wring1"]
                        base_col = ((slot if slot < HALF else slot - HALF) * c.CG
                                    + off * 128)
                        lhsT = wr[:, base_col: base_col + 128]
                        mm = tensor.matmul(
                            psum[:, ti:ti + 1], lhsT,
                            rhs_t[:, rhs_base + ci:rhs_base + ci + 1],
                            start=(start_grp and ci == 0),
                            stop=(stop_grp and ci == C - 1),
                        )
                        last_of_chunk = (off == c.TPC - 1) or (g == n_tiles - 1)
                        if last_of_chunk:
                            mm.then_inc(sem["fsem"], 1)
                    chunk_idx[0] = base + (n_tiles + c.TPC - 1) // c.TPC

                tensor.wait_ge(sem["vsem"], VS_ALL)
                stage(ps["p_attn"], c.Ca, c.Ta, pb, rhs_base=OFF_EH)
                stage(ps["p_enc"], c.Ce, c.Te, sb["exp_sb"],
                      pre_wait=(sem["act"], A1_EXP))
                tensor.matmul(ps["p_S"][0:1, 0:1], onesc_v,
                              sb["sumexp"][:, 0:1]).then_inc(sem["pe"], 1)
                tensor.wait_ge(sem["dve"], DM["recip"])
                tensor.matmul(ps["p_bc"][:, 0:1], sb["ones_r"][0:1, :],
                              sb["recip"][0:1, 0:1]).then_inc(sem["pe"], 1)
                stage(ps["p_comb"], c.Cc, c.Tc, pb, rhs_base=OFF_CB,
                      pre_wait=(sem["dve"], DM["ctx"]))
                stage(ps["p_gh"], c.Cg, c.Tg, pb, rhs_base=OFF_H)
                stage(ps["p_gx"], c.Cg, c.Tg, sb["x_sb"],
                      pre_wait=(sem["act"], A3_RELU))
                stage(ps["p_z"], c.Co, c.Tv, sb["hnew_bf"],
                      pre_wait=(sem["dve"], DM["hnewbf"]))
                tensor.wait_ge(sem["act"], A6_EZ)
                tensor.matmul(ps["p_S"][0:1, 0:1], onesc_v,
                              sb["sez"][:, 0:1]).then_inc(sem["pe"], 1)
                tensor.wait_ge(sem["act"], A7_LN)
                tensor.matmul(ps["p_bc"][:, 0:1], sb["ones_r"][0:1, :],
                              sb["lse_sb"][0:1, 0:1]).then_inc(sem["pe"], 1)

            # ---------------- scalar (ACT) ------------------------------------
            @block.scalar
            def _(s):
                AF = mybir.ActivationFunctionType
                Q = c.Q
                if c.rdma:
                    s.wait_ge(sem["gdone"], 1)
                    s_src = sb["sacc"][:, :]
                else:
                    s.wait_ge(sem["g_ag_out"], 16)
                    s_src = sb["s_all"][:, :]
                s.activation(sb["exp_sb"][:, :], s_src, AF.Exp,
                             accum_out=sb["sumexp"][:, 0:1]).then_inc(sem["act"], 1)
                s.activation(sb["exp32_sb"][:, :], s_src,
                             AF.Exp).then_inc(sem["act"], 1)
                s.wait_ge(sem["dve"], DM["xb"])
                s.activation(sb["x_sb"][:, :], sb["xb_sb"][:, :],
                             AF.Relu).then_inc(sem["act"], 1)
                s.wait_ge(sem["dve"], DM["gb"])
                s.activation(sb["pw_sb"][:, 0:2 * Q], sb["g_sb"][:, 0:2 * Q],
                             AF.Sigmoid)
                s.activation(sb["pw_sb"][:, 2 * Q:3 * Q], sb["g_sb"][:, 2 * Q:3 * Q],
                             AF.Tanh)
                s.activation(sb["pw_sb"][:, 3 * Q:4 * Q], sb["g_sb"][:, 3 * Q:4 * Q],
                             AF.Sigmoid).then_inc(sem["act"], 1)
                s.wait_ge(sem["dve"], DM["cnew"])
                s.activation(sb["tc_sb"][:, :], sb["outbuf"][:, c.OB_C:c.OB_C + Q],
                             AF.Tanh).then_inc(sem["act"], 1)
                s.wait_ge(sem["dve"], DM["zb"])
                s.activation(sb["ez_sb"][:, :], sb["z_sb"][:, :], AF.Exp,
                             accum_out=sb["sez"][:, 0:1]).then_inc(sem["act"], 1)
                s.wait_ge(sem["pe"], PE_SUMZ)
                s.activation(sb["lse_sb"][0:1, 0:1], ps["p_S"][0:1, 0:1],
                             AF.Ln).then_inc(sem["act"], 1)

            # ---------------- gpsimd: collectives -----------------------------
            @block.gpsimd
            def _(gp):
                if c.rdma:
                    from concourse import library_config
                    gp.load_library(library_config.proxy)
                    # Hand-rolled recursive-doubling exchanges over direct
                    # core-to-core SBUF DMA (no ncfw, no entry barrier).
                    # Peer of round r is (me XOR 2^r); rdests slot k carries
                    # Δtpb=k so cross-die dests land on D2D-capable lanes.
                    cnt = {"p": 0, "s": 0}

                    def xsend(dst_ap, src_ap, rsem):
                        rdst = [None] * 8
                        step = 1 << xsend.r
                        rdst[step] = (0, step)
                        gp.remote_dma_broadcast(
                            out_ap=dst_ap, in_ap=src_ap,
                            remote_sem=rsem, local_sem=sem["lsem"],
                            rdests=rdst,
                        ).then_inc(sem["psem"], 1)
                        cnt["p"] += 1
                        cnt["s"] += 1
                        gp.wait_ge(sem["psem"], cnt["p"])
                        gp.trigger_dma(1)

                    # scores all-gather (me-relative block order)
                    gp.wait_ge(sem["dve"], DM["s"])
                    for r in range(3):
                        xsend.r = r
                        w = (1 << r) * c.Ta
                        xsend(sb["sacc"][:, w:2 * w], sb["sacc"][:, 0:w],
                              sem[f"rs_s{r}"])
                        wi = gp.wait_ge(sem[f"rs_s{r}"], 2)
                        if r == 2:
                            wi.then_inc(sem["gdone"], 1)

                    # x all-reduce
                    gp.wait_ge(sem["dve"], DM["ar1"])
                    nadd = 0
                    for r in range(3):
                        xsend.r = r
                        rbuf = sb[f"rx{r}"]
                        xsend(rbuf[:, :], sb["ar1_sb"][:, :], sem[f"rs_x{r}"])
                        gp.wait_ge(sem[f"rs_x{r}"], 2)
                        gp.wait_ge(sem["lsem"], 16 * cnt["s"])
                        add = gp.tensor_add(sb["ar1_sb"][:, :], sb["ar1_sb"][:, :],
                                            rbuf[:, :])
                        nadd += 1
                        add.then_inc(sem["gadd"], 1)
                        gp.wait_ge(sem["gadd"], nadd)
                        if r == 2:
                            gp.engine_nop().then_inc(sem["gdone"], 1)

                    # z all-reduce
                    gp.wait_ge(sem["dve"], DM["ar2"])
                    for r in range(3):
                        xsend.r = r
                        rbuf = sb[f"rz{r}"]
                        xsend(rbuf[:, :], sb["ar2_sb"][:, :], sem[f"rs_z{r}"])
                        gp.wait_ge(sem[f"rs_z{r}"], 2)
                        gp.wait_ge(sem["lsem"], 16 * cnt["s"])
                        add = gp.tensor_add(sb["ar2_sb"][:, :], sb["ar2_sb"][:, :],
                                            rbuf[:, :])
                        nadd += 1
                        add.then_inc(sem["gadd"], 1)
                        gp.wait_ge(sem["gadd"], nadd)
                        if r == 2:
                            gp.engine_nop().then_inc(sem["gdone"], 1)
                    return

                rg = [core_ids]
                # warmup collective: fires the ncfw entry barrier at t=0 so
                # the real collectives below see a warm collective engine
                if c.warm:
                    gp.dma_start(out=warm_in[:],
                                 in_=dparams["ones_row"][0:1, 0:8]).then_inc(
                        sem["g_warm"], 16)
                    gp.wait_ge(sem["g_warm"], 16)
                    gp.collective_compute(
                        "AllGather", mybir.AluOpType.bypass, replica_groups=rg,
                        ins=[warm_in[:]], outs=[warm_out[:]],
                    ).then_inc(sem["gc"], 1)
                GCW = 1 if c.warm else 0
                # scores all-gather
                gp.wait_ge(sem["dve"], DM["s"])
                ag_in_ap = bass.AP(ag_in, 0, [[c.Ta, 128], [1, c.Ta]])
                gp.dma_start(out=ag_in_ap, in_=sb["s_sb"][:, :]).then_inc(
                    sem["g_ag_in"], 16)
                gp.wait_ge(sem["g_ag_in"], 16)
                gp.collective_compute(
                    "AllGather", mybir.AluOpType.bypass, replica_groups=rg,
                    ins=[ag_in[:]], outs=[ag_out[:, :, :]],
                ).then_inc(sem["gc"], 1)
                gp.wait_ge(sem["gc"], GCW + 1)
                # gather back: s_all[p, (k,t)] = ag_out[k, p, t]
                src = bass.AP(ag_out, 0,
                              [[c.Ta, 128], [128 * c.Ta, c.NC], [1, c.Ta]])
                dst = sb["s_all"][:, :].rearrange("p (k t) -> p k t", k=c.NC)
                with nc.allow_non_contiguous_dma(reason="8KB score gather"):
                    gp.dma_start(out=dst, in_=src).then_inc(sem["g_ag_out"], 16)
                # x all-reduce
                gp.wait_ge(sem["dve"], DM["ar1"])
                gp.dma_start(out=ar1_in[:, :], in_=sb["ar1_sb"][:, :]).then_inc(
                    sem["g_ar1_in"], 16)
                gp.wait_ge(sem["g_ar1_in"], 16)
                gp.collective_compute(
                    "AllReduce", mybir.AluOpType.add, replica_groups=rg,
                    ins=[ar1_in[:, :]], outs=[ar1_out[:, :]],
                ).then_inc(sem["gc"], 1)
                gp.wait_ge(sem["gc"], GCW + 2)
                gp.dma_start(out=sb["x_raw"][:, :], in_=ar1_out[:, :]).then_inc(
                    sem["g_ar1_out"], 16)
                # z all-reduce
                gp.wait_ge(sem["dve"], DM["ar2"])
                gp.dma_start(out=ar2_in[:, :], in_=sb["ar2_sb"][:, :]).then_inc(
                    sem["g_ar2_in"], 16)
                gp.wait_ge(sem["g_ar2_in"], 16)
                gp.collective_compute(
                    "AllReduce", mybir.AluOpType.add, replica_groups=rg,
                    ins=[ar2_in[:, :]], outs=[ar2_out[:, :]],
                ).then_inc(sem["gc"], 1)
                gp.wait_ge(sem["gc"], GCW + 3)
                gp.dma_start(out=sb["z_raw"][:, :], in_=ar2_out[:, :]).then_inc(
                    sem["g_ar2_out"], 16)

    # populate .instr bytes for extended-inst ISA subclasses (remote DMA,
    # library reload) — raw Bass skips this pass and walrus then rejects the
    # empty instr with "ISA wrong length"
    mybir.codegen_inst_isa_subclasses(nc)
    return nc


# ---------------------------------------------------------------------------
# entry point
# ---------------------------------------------------------------------------

_NC_CACHE = {}


def _get_nc(c: Cfg):
    key = (c.H, c.L, c.NC, c.RING, c.TPC)
    if key not in _NC_CACHE:
        _NC_CACHE[key] = build_nc(c)
    return _NC_CACHE[key]


def run(inputs: dict, trace: bool = False, tmpdir: str | None = None, cfg: Cfg = CFG):
    from concourse.bass_utils import run_bass_kernel_spmd

    in_maps = prep_inputs(cfg, inputs)
    nc = _get_nc(cfg)
    res = run_bass_kernel_spmd(nc, in_maps, list(range(cfg.NC)), trace=trace,
                               tmpdir=tmpdir)
    outs = decode_outputs(cfg, [r["out"] for r in res.results])
    return outs, res


def kernel(**inputs):
    outs, _ = run(inputs, trace=bool(os.environ.get("KERNEL_TRACE")))
    return outs


# revision 40
# speedup vs baseline: 36.1549x; 36.1549x over previous
"""AttentionDecoder single-step kernel for 8 TRN2 NeuronCores.

Math (see reference):
    e      = emb[token]                                   (E,)
    s      = [e,h] @ W_attn.T + b_attn                    (L,)
    a      = softmax(s)                                   (L,)
    ctx    = a @ enc                                      (H,)
    x      = relu([e,ctx] @ W_comb.T + b_comb)            (H,)
    gates  = x @ W_ih.T + h @ W_hh.T + b_ih + b_hh        (4H,)
    c'     = sig(f)*c + sig(i)*tanh(g)
    h'     = sig(o)*tanh(c')
    logp   = log_softmax(h' @ W_out.T + b_out)            (V,)

Everything is a batch-1 matvec, so the kernel is HBM-bandwidth bound on the
~833 MB of weights.  Sharding (8 cores):
    W_attn  row (output L) sharded     -> AllGather of the 256 scores/core
    enc     column (H) sharded         -> ctx slice per core
    W_comb  input (contraction) shard  -> AllReduce of partial x
    W_ih/hh row (gate dim) sharded     -> local c'/h' slices
    W_out   input (h) sharded          -> AllReduce of partial logits z

Device layout convention: every length-N vector lives in SBUF as
[128, N/128] with element (p, c) = v[c*128 + p] ("chunk-per-column"), so it
is directly usable as matmul lhsT/rhs chunks and all pointwise ops run
128-lanes wide.  Weights are pre-transposed and pre-tiled on the host into
one flat [128, tiles*128] blob per stage, streamed through an SBUF ring by
the sync engine while the PE consumes [128,128] stationary tiles.
"""

import os
from dataclasses import dataclass, field

import numpy as np


# ---------------------------------------------------------------------------
# configuration
# ---------------------------------------------------------------------------

@dataclass(frozen=True)
class Cfg:
    H: int = 4096          # hidden = embedding = vocab
    L: int = 2048          # encoder length
    NC: int = 8            # cores
    RING: int = 16         # weight ring slots
    TPC: int = 32          # tiles (128x128) per DMA chunk
    wdt: str = "bf16"      # matmul operand dtype: "bf16" or "f32"
    warm: bool = False     # warmup collective deadlocks ncfw; keep off
    rdma: bool = False     # remote-DMA collectives: no broadcast ucode on this fleet
    early_hh: bool = True  # stream W_hh right after W_attn to hide AG latency

    @property
    def EH(self):
        return 2 * self.H

    @property
    def L_sh(self):
        return self.L // self.NC

    @property
    def H_sh(self):
        return self.H // self.NC

    # per-stage (contraction chunks C, output tiles T)
    @property
    def Ca(self):
        return self.EH // 128

    @property
    def Ta(self):
        return self.L_sh // 128

    @property
    def Ce(self):
        return self.L // 128

    @property
    def Te(self):
        return self.H_sh // 128

    @property
    def Cc(self):
        return (2 * self.H_sh) // 128

    @property
    def Tc(self):
        return self.H // 128

    @property
    def Cg(self):
        return self.H // 128

    @property
    def Tg(self):
        return (4 * self.H_sh) // 128

    @property
    def Q(self):
        return self.Tg // 4

    @property
    def Co(self):
        return self.H_sh // 128

    @property
    def Tv(self):
        return self.H // 128

    @property
    def Tla(self):
        return self.NC * self.Ta  # attn prob columns (full L)

    # outbuf column offsets
    @property
    def OB_H(self):
        return self.Tv

    @property
    def OB_C(self):
        return self.Tv + self.Q

    @property
    def OB_A(self):
        return self.Tv + 2 * self.Q

    @property
    def OBW(self):
        return self.Tv + 2 * self.Q + self.Tla

    @property
    def CG(self):
        return self.TPC * 128  # ring slot columns


CFG = Cfg()


# stage table: (name, C, T) -- order == stream order.  w_hh only needs h
# (available at t=0), so streaming it early hides the collective-entry
# barrier + score-AllGather latency that stalls w_enc.
def _stages(c: Cfg):
    if c.early_hh:
        return [
            ("w_attn", c.Ca, c.Ta),
            ("w_hh", c.Cg, c.Tg),
            ("w_enc", c.Ce, c.Te),
            ("w_comb", c.Cc, c.Tc),
            ("w_ih", c.Cg, c.Tg),
            ("w_out", c.Co, c.Tv),
        ]
    return [
        ("w_attn", c.Ca, c.Ta),
        ("w_enc", c.Ce, c.Te),
        ("w_comb", c.Cc, c.Tc),
        ("w_hh", c.Cg, c.Tg),
        ("w_ih", c.Cg, c.Tg),
        ("w_out", c.Co, c.Tv),
    ]


# ---------------------------------------------------------------------------
# host-side input prep
# ---------------------------------------------------------------------------

def _np_wdt(c):
    if c.wdt == "bf16":
        import ml_dtypes
        return ml_dtypes.bfloat16
    return np.float32


def _pack(arr_km: np.ndarray, C: int, T: int, dtype=np.float32) -> np.ndarray:
    """[C*128, T*128] lhsT matrix -> [128, C*T*128] tile stream.

    Tile order is (t outer, c inner) -- K-contiguous so each PSUM column's
    accumulation group closes before the next one starts (one open group per
    PSUM bank).  tile (t,c)[p,m] = arr[c*128+p, t*128+m].
    """
    assert arr_km.shape == (C * 128, T * 128), (arr_km.shape, C, T)
    return np.ascontiguousarray(
        arr_km.reshape(C, 128, T, 128).transpose(1, 2, 0, 3).reshape(128, C * T * 128),
        dtype=dtype,
    )


# Logical->physical NC map on TRN2 (driver BASE table).  The recursive-
# doubling exchange pairs cores by XOR on *physical* ids, so after the
# 3 rounds, score block j on core k holds the scores of logical core
# k ^ _GPERM[j].  The per-device constant XOR cancels out of the pairing.
_GPERM = (0, 1, 2, 3, 6, 7, 4, 5)


def _cols(v: np.ndarray, dtype=np.float32) -> np.ndarray:
    """length-N vector -> [128, N/128] chunk-per-column layout."""
    n = v.shape[0]
    assert n % 128 == 0
    return np.ascontiguousarray(v.reshape(n // 128, 128).T, dtype=dtype)


def prep_inputs(c: Cfg, inputs: dict) -> list[dict]:
    H, L, NC = c.H, c.L, c.NC
    token = np.asarray(inputs["token"])
    h0 = np.asarray(inputs["h"], np.float32)[0]
    c0 = np.asarray(inputs["c"], np.float32)[0]
    enc = np.asarray(inputs["encoder_outputs"], np.float32)
    emb = np.asarray(inputs["emb"], np.float32)
    W_attn = np.asarray(inputs["W_attn"], np.float32)
    b_attn = np.asarray(inputs["b_attn"], np.float32)
    W_comb = np.asarray(inputs["W_comb"], np.float32)
    b_comb = np.asarray(inputs["b_comb"], np.float32)
    W_ih = np.asarray(inputs["W_ih"], np.float32)
    W_hh = np.asarray(inputs["W_hh"], np.float32)
    b_ih = np.asarray(inputs["b_ih"], np.float32)
    b_hh = np.asarray(inputs["b_hh"], np.float32)
    W_out = np.asarray(inputs["W_out"], np.float32)
    b_out = np.asarray(inputs["b_out"], np.float32)

    e = emb[int(token.reshape(-1)[0])]
    eh = np.concatenate([e, h0])
    b_g = b_ih + b_hh

    W_attn_T = np.ascontiguousarray(W_attn.T)        # [2H, L]
    W_comb_T = np.ascontiguousarray(W_comb.T)        # [2H, H]
    W_ih_T = np.ascontiguousarray(W_ih.T)            # [H, 4H]
    W_hh_T = np.ascontiguousarray(W_hh.T)            # [H, 4H]
    W_out_T = np.ascontiguousarray(W_out.T)          # [H, V]

    in_maps = []
    for k in range(NC):
        ls = slice(k * c.L_sh, (k + 1) * c.L_sh)
        hs = slice(k * c.H_sh, (k + 1) * c.H_sh)
        # gate-dim columns of W_*_T for this core: (gate, u, p) order
        gcols = np.concatenate(
            [np.arange(g * H + k * c.H_sh, g * H + (k + 1) * c.H_sh) for g in range(4)]
        )
        wt = _np_wdt(c)
        m = {
            "pack_bf": np.concatenate(
                [_cols(eh, wt), _cols(h0, wt), _cols(e[hs], wt)], axis=1),
            "pack_f32": np.concatenate(
                [_cols(c0[hs]), _cols(b_attn[ls]), _cols(b_comb),
                 _cols(b_g[gcols]), _cols(b_out),
                 np.ones((128, 1), np.float32)], axis=1),
            "ones_row": np.ones((1, 128), np.float32),
            "w_attn": _pack(W_attn_T[:, ls], c.Ca, c.Ta, wt),
            "w_enc": _pack(
                np.concatenate(
                    [enc[(k ^ _GPERM[j]) * c.L_sh:((k ^ _GPERM[j]) + 1) * c.L_sh,
                         hs] for j in range(NC)]) if c.rdma else enc[:, hs],
                c.Ce, c.Te, wt),
            "w_comb": _pack(
                np.concatenate([W_comb_T[hs, :], W_comb_T[H + k * c.H_sh:
                                                          H + (k + 1) * c.H_sh, :]]),
                c.Cc, c.Tc, wt),
            "w_hh": _pack(W_hh_T[:, gcols], c.Cg, c.Tg, wt),
            "w_ih": _pack(W_ih_T[:, gcols], c.Cg, c.Tg, wt),
            "w_out": _pack(W_out_T[hs, :], c.Co, c.Tv, wt),
        }
        in_maps.append(m)
    return in_maps


def decode_outputs(c: Cfg, outs: list[np.ndarray]):
    """per-core [128, OBW] f32 -> (logp, h_new, c_new, attn_w) full tensors."""
    o0 = np.asarray(outs[0], np.float32)
    logp = o0[:, 0:c.Tv].T.reshape(-1)
    attn = o0[:, c.OB_A:c.OB_A + c.Tla].T.reshape(-1)
    if c.rdma:
        # core 0's score block j holds logical core _GPERM[j]'s slice
        blocks = attn.reshape(c.NC, c.L_sh)
        attn = np.concatenate([blocks[_GPERM[q]] for q in range(c.NC)])
    h_new = np.concatenate(
        [np.asarray(outs[k], np.float32)[:, c.OB_H:c.OB_H + c.Q].T.reshape(-1)
         for k in range(c.NC)]
    )
    c_new = np.concatenate(
        [np.asarray(outs[k], np.float32)[:, c.OB_C:c.OB_C + c.Q].T.reshape(-1)
         for k in range(c.NC)]
    )
    return (logp[None, :], h_new[None, :], c_new[None, :], attn[None, :])


# ---------------------------------------------------------------------------
# device graph
# ---------------------------------------------------------------------------

def build_nc(c: Cfg):
    import concourse.bass as bass
    import concourse.mybir as mybir
    from contextlib import ExitStack

    f32 = mybir.dt.float32
    wdt = mybir.dt.bfloat16 if c.wdt == "bf16" else mybir.dt.float32
    nc = bass.Bass(target_bir_lowering=False)
    core_ids = list(range(c.NC))

    # ---- DRAM parameters ------------------------------------------------
    # packed small params: one bf16 pack, one f32 pack, plus ones_row
    PBF = c.Ca + c.Cg + c.Cc // 2                 # eh | h | e_comb
    PF32 = c.Q + c.Ta + c.Tc + c.Tg + c.Tv + 1    # c | ba | bc | bg | bo | ones
    small = {
        "pack_bf": [128, PBF], "pack_f32": [128, PF32], "ones_row": [1, 128],
    }
    dparams = {}
    dparams["pack_bf"] = nc.declare_dram_parameter("pack_bf", [128, PBF], wdt,
                                                   isOutput=False)
    dparams["pack_f32"] = nc.declare_dram_parameter("pack_f32", [128, PF32], f32,
                                                    isOutput=False)
    dparams["ones_row"] = nc.declare_dram_parameter("ones_row", [1, 128], f32,
                                                    isOutput=False)
    for name, C, T in _stages(c):
        dparams[name] = nc.declare_dram_parameter(
            name, [128, C * T * 128], wdt, isOutput=False)
    out_ext = nc.declare_dram_parameter("out", [128, c.OBW], f32, isOutput=True)

    # collective bounce buffers (internal DRAM; outs must be Shared)
    warm_in = nc.dram_tensor("warm_in", [8], f32)
    warm_out = nc.dram_tensor("warm_out", [8 * c.NC], f32, addr_space="Shared")
    ag_in = nc.dram_tensor("ag_in", [c.L_sh], f32)
    ag_out = nc.dram_tensor("ag_out", [c.NC, 128, c.Ta], f32, addr_space="Shared")
    ar1_in = nc.dram_tensor("ar1_in", [128, c.Tc], f32)
    ar1_out = nc.dram_tensor("ar1_out", [128, c.Tc], f32, addr_space="Shared")
    ar2_in = nc.dram_tensor("ar2_in", [128, c.Tv], f32)
    ar2_out = nc.dram_tensor("ar2_out", [128, c.Tv], f32, addr_space="Shared")

    # chunk plan: (param_name, col_start, col_end) in stream order
    chunks = []
    F = {}  # stage name -> cumulative chunk count when stage fully consumed
    for name, C, T in _stages(c):
        total = C * T
        g = 0
        while g < total:
            g2 = min(g + c.TPC, total)
            chunks.append((name, g * 128, g2 * 128))
            g += c.TPC
        F[name] = len(chunks)

    ctx = ExitStack()
    with ctx:
        HALF = (c.RING + 1) // 2
        sb = {}
        for name, (shape, dt_) in {
            "wring0": ([128, HALF * c.CG], wdt),
            "wring1": ([128, (c.RING - HALF) * c.CG], wdt),
            "pbf_sb": ([128, c.Ca + c.Cg + c.Cc], wdt),
            "pf32_sb": ([128, PF32], f32),
            "x_sb": ([128, c.Cg], wdt),
            "xb_sb": ([128, c.Cg], f32),
            "s_sb": ([128, c.Ta], f32), "s_all": ([128, c.Tla], f32),
            "sacc": ([128, c.Tla], f32),
            "rx0": ([128, c.Tc], f32), "rx1": ([128, c.Tc], f32),
            "rx2": ([128, c.Tc], f32),
            "rz0": ([128, c.Tv], f32), "rz1": ([128, c.Tv], f32),
            "rz2": ([128, c.Tv], f32),
            "exp_sb": ([128, c.Tla], wdt), "exp32_sb": ([128, c.Tla], f32),
            "sumexp": ([128, 1], f32), "recip": ([128, 1], f32),
            "rs_sb": ([128, 1], f32),
            "ar1_sb": ([128, c.Tc], f32), "x_raw": ([128, c.Tc], f32),
            "g_sb": ([128, c.Tg], f32), "g1_sb": ([128, c.Tg], f32),
            "pw_sb": ([128, c.Tg], f32),
            "tc_sb": ([128, c.Q], f32), "t1_sb": ([128, c.Q], f32),
            "t2_sb": ([128, c.Q], f32), "hnew_bf": ([128, c.Q], wdt),
            "ar2_sb": ([128, c.Tv], f32), "z_raw": ([128, c.Tv], f32),
            "z_sb": ([128, c.Tv], f32),
            "ez_sb": ([128, c.Tv], f32), "sez": ([128, 1], f32),
            "lse_sb": ([128, 1], f32), "lbc_sb": ([128, 1], f32),
            "ones_r": ([1, 128], f32),
            "outbuf": ([128, c.OBW], f32),
        }.items():
            sb[name] = ctx.enter_context(nc.sbuf_tensor(name, shape, dt_))

        # views into the packed tensors (plain AP slices)
        pb, pf = sb["pbf_sb"], sb["pf32_sb"]
        o = [0]

        def _take(pack, n):
            a, o[0] = o[0], o[0] + n
            return pack[:, a:a + n]

        eh_v = _take(pb, c.Ca)
        h_v = _take(pb, c.Cg)
        comb_v = pb[:, o[0]:o[0] + c.Cc]   # e_comb cols + ctx cols
        o[0] = 0
        c_v = _take(pf, c.Q)
        battn_v = _take(pf, c.Ta)
        bcomb_v = _take(pf, c.Tc)
        bgate_v = _take(pf, c.Tg)
        bout_v = _take(pf, c.Tv)
        onesc_v = _take(pf, 1)

        ps = {}
        for name, shape in {
            "p_attn": [128, c.Ta], "p_enc": [128, c.Te], "p_comb": [128, c.Tc],
            "p_gh": [128, c.Tg], "p_gx": [128, c.Tg], "p_z": [128, c.Tv],
            "p_S": [128, 1], "p_bc": [128, 1],
        }.items():
            ps[name] = ctx.enter_context(nc.psum_tensor(name, shape, f32))

        sem = {}
        for name in ["vsem", "fsem", "pe", "dve", "act", "gc", "odsem", "g_warm",
                     "g_ag_in", "g_ag_out", "g_ar1_in", "g_ar1_out",
                     "g_ar2_in", "g_ar2_out",
                     "psem", "lsem", "gdone", "gadd",
                     "rs_s0", "rs_s1", "rs_s2", "rs_x0", "rs_x1", "rs_x2",
                     "rs_z0", "rs_z1", "rs_z2"]:
            sem[name] = ctx.enter_context(nc.semaphore(name))
        dsl = [ctx.enter_context(nc.semaphore(f"dsl{i}")) for i in range(c.RING)]

        # milestone counts
        N_SMALL = len(small)
        VS_ALL = 16 * N_SMALL
        PE_SUMS, PE_BC, PE_SUMZ, PE_LSEBC = range(1, 5)
        (A1_EXP, A2_EXP32, A3_RELU, A4_PW, A5_TANHC, A6_EZ,
         A7_LN) = range(1, 8)
        D1_S, D2_RECIP, D3_CTX, D4_AR1, D5_XB, D6_GB, D7_CNEW, D8_HNEW, \
            D9_AR2, D10_ZB, D11_LOGP = range(1, 12)

        small_src = {  # sbuf destination of each small param
            "pack_bf": pb[:, 0:PBF], "pack_f32": pf[:, :],
            "ones_row": sb["ones_r"][:, :],
        }
        OFF_EH, OFF_H, OFF_CB = 0, c.Ca, c.Ca + c.Cg

        with nc.Block() as block:

            DM = {}  # dve milestone name -> count
            dve_n = [0]

            def dinc(inst, name=None):
                dve_n[0] += 1
                inst.then_inc(sem["dve"], 1)
                if name:
                    DM[name] = dve_n[0]

            # ---------------- vector (DVE) -----------------------------------
            # emitted first so other engines can reference DM[...] counts
            @block.vector
            def _(v):
                Q = c.Q
                v.wait_ge(sem["fsem"], F["w_attn"])
                s_dst = sb["sacc"][:, 0:c.Ta] if c.rdma else sb["s_sb"][:, :]
                dinc(v.tensor_add(s_dst, ps["p_attn"][:, :], battn_v), "s")
                v.wait_ge(sem["pe"], PE_SUMS)
                dinc(v.reciprocal(sb["recip"][0:1, 0:1],
                                  ps["p_S"][0:1, 0:1]), "recip")
                v.wait_ge(sem["fsem"], F["w_enc"])
                v.wait_ge(sem["pe"], PE_BC)
                dinc(v.tensor_copy(sb["rs_sb"][:, :], ps["p_bc"][:, :]), "rs")
                v.wait_ge(sem["dve"], DM["rs"])
                dinc(v.tensor_scalar_mul(
                    pb[:, OFF_CB + c.Cc // 2:OFF_CB + c.Cc], ps["p_enc"][:, :],
                    sb["rs_sb"][:, 0:1]), "ctx")
                v.wait_ge(sem["act"], A2_EXP32)
                v.tensor_scalar_mul(
                    sb["outbuf"][:, c.OB_A:c.OB_A + c.Tla], sb["exp32_sb"][:, :],
                    sb["rs_sb"][:, 0:1])
                v.wait_ge(sem["fsem"], F["w_comb"])
                dinc(v.tensor_copy(sb["ar1_sb"][:, :], ps["p_comb"][:, :]), "ar1")
                if c.rdma:
                    v.wait_ge(sem["gdone"], 2)
                    x_src = sb["ar1_sb"][:, :]
                else:
                    v.wait_ge(sem["g_ar1_out"], 16)
                    x_src = sb["x_raw"][:, :]
                dinc(v.tensor_add(sb["xb_sb"][:, :], x_src, bcomb_v), "xb")
                v.wait_ge(sem["fsem"], F["w_ih"])
                dinc(v.tensor_add(sb["g1_sb"][:, :], ps["p_gh"][:, :],
                                  bgate_v), "g1")
                v.wait_ge(sem["dve"], DM["g1"])
                dinc(v.tensor_add(sb["g_sb"][:, :], sb["g1_sb"][:, :],
                                  ps["p_gx"][:, :]), "gb")
                v.wait_ge(sem["act"], A4_PW)
                v.tensor_mul(sb["t1_sb"][:, :], sb["pw_sb"][:, Q:2 * Q], c_v)
                dinc(v.tensor_mul(sb["t2_sb"][:, :], sb["pw_sb"][:, 0:Q],
                                  sb["pw_sb"][:, 2 * Q:3 * Q]), "t2")
                v.wait_ge(sem["dve"], DM["t2"])
                dinc(v.tensor_add(sb["outbuf"][:, c.OB_C:c.OB_C + Q],
                                  sb["t1_sb"][:, :], sb["t2_sb"][:, :]), "cnew")
                v.wait_ge(sem["act"], A5_TANHC)
                dinc(v.tensor_mul(sb["outbuf"][:, c.OB_H:c.OB_H + Q],
                                  sb["pw_sb"][:, 3 * Q:4 * Q],
                                  sb["tc_sb"][:, :]), "hnew")
                v.wait_ge(sem["dve"], DM["hnew"])
                dinc(v.tensor_copy(sb["hnew_bf"][:, :],
                                   sb["outbuf"][:, c.OB_H:c.OB_H + Q]), "hnewbf")
                v.wait_ge(sem["fsem"], F["w_out"])
                dinc(v.tensor_copy(sb["ar2_sb"][:, :], ps["p_z"][:, :]), "ar2")
                if c.rdma:
                    v.wait_ge(sem["gdone"], 3)
                    z_src = sb["ar2_sb"][:, :]
                else:
                    v.wait_ge(sem["g_ar2_out"], 16)
                    z_src = sb["z_raw"][:, :]
                dinc(v.tensor_add(sb["z_sb"][:, :], z_src, bout_v), "zb")
                v.wait_ge(sem["pe"], PE_LSEBC)
                dinc(v.tensor_copy(sb["lbc_sb"][:, :], ps["p_bc"][:, :]), "lbc")
                v.wait_ge(sem["dve"], DM["lbc"])
                dinc(v.tensor_scalar(
                    out=sb["outbuf"][:, 0:c.Tv], in0=sb["z_sb"][:, :],
                    scalar1=sb["lbc_sb"][:, 0:1], scalar2=None,
                    op0=mybir.AluOpType.subtract), "logp")

            # ---------------- sync: all the big streaming DMA ----------------
            @block.sync
            def _(sync):
                for name in small:
                    sync.dma_start(
                        out=small_src[name], in_=dparams[name][:, :]
                    ).then_inc(sem["vsem"], 16)
                for i, (name, c0_, c1_) in enumerate(chunks):
                    if i >= c.RING:
                        sync.wait_ge(sem["fsem"], i - c.RING + 1)
                    slot = i % c.RING
                    wr = sb["wring0"] if slot < HALF else sb["wring1"]
                    off = (slot if slot < HALF else slot - HALF) * c.CG
                    sync.dma_start(
                        out=wr[:, off:off + (c1_ - c0_)],
                        in_=dparams[name][:, c0_:c1_],
                    ).then_inc(dsl[slot], 16)
                sync.wait_ge(sem["dve"], DM["logp"])
                sync.dma_start(out=out_ext[:, :], in_=sb["outbuf"][:, :]).then_inc(
                    sem["odsem"], 16)
                sync.wait_ge(sem["odsem"], 16)

            # ---------------- tensor: every matmul ---------------------------
            @block.tensor
            def _(tensor):
                chunk_idx = [0]  # mutable closure counter

                def stage(psum, C, T, rhs_t, rhs_base=0, start_grp=True,
                          stop_grp=True, pre_wait=None):
                    if pre_wait is not None:
                        tensor.wait_ge(*pre_wait)
                    base = chunk_idx[0]
                    n_tiles = C * T
                    for g in range(n_tiles):
                        ti, ci = divmod(g, C)
                        local_chunk, off = divmod(g, c.TPC)
                        gl_chunk = base + local_chunk
                        if off == 0:
                            tensor.wait_ge(dsl[gl_chunk % c.RING],
                                           16 * (gl_chunk // c.RING + 1))
                        slot = gl_chunk % c.RING
                        wr = sb["wring0"] if slot < HALF else sb["wring1"]
                        base_col = ((slot if slot < HALF else slot - HALF) * c.CG
                                    + off * 128)
                        lhsT = wr[:, base_col: base_col + 128]
                        mm = tensor.matmul(
                            psum[:, ti:ti + 1], lhsT,
                            rhs_t[:, rhs_base + ci:rhs_base + ci + 1],
                            start=(start_grp and ci == 0),
                            stop=(stop_grp and ci == C - 1),
                        )
                        last_of_chunk = (off == c.TPC - 1) or (g == n_tiles - 1)
                        if last_of_chunk:
                            mm.then_inc(sem["fsem"], 1)
                    chunk_idx[0] = base + (n_tiles + c.TPC - 1) // c.TPC

                tensor.wait_ge(sem["vsem"], VS_ALL)
                stage(ps["p_attn"], c.Ca, c.Ta, pb, rhs_base=OFF_EH)
                if c.early_hh:
                    stage(ps["p_gh"], c.Cg, c.Tg, pb, rhs_base=OFF_H)
                stage(ps["p_enc"], c.Ce, c.Te, sb["exp_sb"],
                      pre_wait=(sem["act"], A1_EXP))
                tensor.matmul(ps["p_S"][0:1, 0:1], onesc_v,
                              sb["sumexp"][:, 0:1]).then_inc(sem["pe"], 1)
                tensor.wait_ge(sem["dve"], DM["recip"])
                tensor.matmul(ps["p_bc"][:, 0:1], sb["ones_r"][0:1, :],
                              sb["recip"][0:1, 0:1]).then_inc(sem["pe"], 1)
                stage(ps["p_comb"], c.Cc, c.Tc, pb, rhs_base=OFF_CB,
                      pre_wait=(sem["dve"], DM["ctx"]))
                if not c.early_hh:
                    stage(ps["p_gh"], c.Cg, c.Tg, pb, rhs_base=OFF_H)
                stage(ps["p_gx"], c.Cg, c.Tg, sb["x_sb"],
                      pre_wait=(sem["act"], A3_RELU))
                stage(ps["p_z"], c.Co, c.Tv, sb["hnew_bf"],
                      pre_wait=(sem["dve"], DM["hnewbf"]))
                tensor.wait_ge(sem["act"], A6_EZ)
                tensor.matmul(ps["p_S"][0:1, 0:1], onesc_v,
                              sb["sez"][:, 0:1]).then_inc(sem["pe"], 1)
                tensor.wait_ge(sem["act"], A7_LN)
                tensor.matmul(ps["p_bc"][:, 0:1], sb["ones_r"][0:1, :],
                              sb["lse_sb"][0:1, 0:1]).then_inc(sem["pe"], 1)

            # ---------------- scalar (ACT) ------------------------------------
            @block.scalar
            def _(s):
                AF = mybir.ActivationFunctionType
                Q = c.Q
                if c.rdma:
                    s.wait_ge(sem["gdone"], 1)
                    s_src = sb["sacc"][:, :]
                else:
                    s.wait_ge(sem["g_ag_out"], 16)
                    s_src = sb["s_all"][:, :]
                s.activation(sb["exp_sb"][:, :], s_src, AF.Exp,
                             accum_out=sb["sumexp"][:, 0:1]).then_inc(sem["act"], 1)
                s.activation(sb["exp32_sb"][:, :], s_src,
                             AF.Exp).then_inc(sem["act"], 1)
                s.wait_ge(sem["dve"], DM["xb"])
                s.activation(sb["x_sb"][:, :], sb["xb_sb"][:, :],
                             AF.Relu).then_inc(sem["act"], 1)
                s.wait_ge(sem["dve"], DM["gb"])
                s.activation(sb["pw_sb"][:, 0:2 * Q], sb["g_sb"][:, 0:2 * Q],
                             AF.Sigmoid)
                s.activation(sb["pw_sb"][:, 2 * Q:3 * Q], sb["g_sb"][:, 2 * Q:3 * Q],
                             AF.Tanh)
                s.activation(sb["pw_sb"][:, 3 * Q:4 * Q], sb["g_sb"][:, 3 * Q:4 * Q],
                             AF.Sigmoid).then_inc(sem["act"], 1)
                s.wait_ge(sem["dve"], DM["cnew"])
                s.activation(sb["tc_sb"][:, :], sb["outbuf"][:, c.OB_C:c.OB_C + Q],
                             AF.Tanh).then_inc(sem["act"], 1)
                s.wait_ge(sem["dve"], DM["zb"])
                s.activation(sb["ez_sb"][:, :], sb["z_sb"][:, :], AF.Exp,
                             accum_out=sb["sez"][:, 0:1]).then_inc(sem["act"], 1)
                s.wait_ge(sem["pe"], PE_SUMZ)
                s.activation(sb["lse_sb"][0:1, 0:1], ps["p_S"][0:1, 0:1],
                             AF.Ln).then_inc(sem["act"], 1)

            # ---------------- gpsimd: collectives -----------------------------
            @block.gpsimd
            def _(gp):
                if c.rdma:
                    from concourse import library_config
                    gp.load_library(library_config.proxy)
                    # Hand-rolled recursive-doubling exchanges over direct
                    # core-to-core SBUF DMA (no ncfw, no entry barrier).
                    # Peer of round r is (me XOR 2^r); rdests slot k carries
                    # Δtpb=k so cross-die dests land on D2D-capable lanes.
                    cnt = {"p": 0, "s": 0}

                    def xsend(dst_ap, src_ap, rsem):
                        rdst = [None] * 8
                        step = 1 << xsend.r
                        rdst[step] = (0, step)
                        gp.remote_dma_broadcast(
                            out_ap=dst_ap, in_ap=src_ap,
                            remote_sem=rsem, local_sem=sem["lsem"],
                            rdests=rdst,
                        ).then_inc(sem["psem"], 1)
                        cnt["p"] += 1
                        cnt["s"] += 1
                        gp.wait_ge(sem["psem"], cnt["p"])
                        gp.trigger_dma(1)

                    # scores all-gather (me-relative block order)
                    gp.wait_ge(sem["dve"], DM["s"])
                    for r in range(3):
                        xsend.r = r
                        w = (1 << r) * c.Ta
                        xsend(sb["sacc"][:, w:2 * w], sb["sacc"][:, 0:w],
                              sem[f"rs_s{r}"])
                        wi = gp.wait_ge(sem[f"rs_s{r}"], 2)
                        if r == 2:
                            wi.then_inc(sem["gdone"], 1)

                    # x all-reduce
                    gp.wait_ge(sem["dve"], DM["ar1"])
                    nadd = 0
                    for r in range(3):
                        xsend.r = r
                        rbuf = sb[f"rx{r}"]
                        xsend(rbuf[:, :], sb["ar1_sb"][:, :], sem[f"rs_x{r}"])
                        gp.wait_ge(sem[f"rs_x{r}"], 2)
                        gp.wait_ge(sem["lsem"], 16 * cnt["s"])
                        add = gp.tensor_add(sb["ar1_sb"][:, :], sb["ar1_sb"][:, :],
                                            rbuf[:, :])
                        nadd += 1
                        add.then_inc(sem["gadd"], 1)
                        gp.wait_ge(sem["gadd"], nadd)
                        if r == 2:
                            gp.engine_nop().then_inc(sem["gdone"], 1)

                    # z all-reduce
                    gp.wait_ge(sem["dve"], DM["ar2"])
                    for r in range(3):
                        xsend.r = r
                        rbuf = sb[f"rz{r}"]
                        xsend(rbuf[:, :], sb["ar2_sb"][:, :], sem[f"rs_z{r}"])
                        gp.wait_ge(sem[f"rs_z{r}"], 2)
                        gp.wait_ge(sem["lsem"], 16 * cnt["s"])
                        add = gp.tensor_add(sb["ar2_sb"][:, :], sb["ar2_sb"][:, :],
                                            rbuf[:, :])
                        nadd += 1
                        add.then_inc(sem["gadd"], 1)
                        gp.wait_ge(sem["gadd"], nadd)
                        if r == 2:
                            gp.engine_nop().then_inc(sem["gdone"], 1)
                    return

                rg = [core_ids]
                # warmup collective: fires the ncfw entry barrier at t=0 so
                # the real collectives below see a warm collective engine
                if c.warm:
                    gp.dma_start(out=warm_in[:],
                                 in_=dparams["ones_row"][0:1, 0:8]).then_inc(
                        sem["g_warm"], 16)
                    gp.wait_ge(sem["g_warm"], 16)
                    gp.collective_compute(
                        "AllGather", mybir.AluOpType.bypass, replica_groups=rg,
                        ins=[warm_in[:]], outs=[warm_out[:]],
                    ).then_inc(sem["gc"], 1)
                GCW = 1 if c.warm else 0
                # scores all-gather
                gp.wait_ge(sem["dve"], DM["s"])
                ag_in_ap = bass.AP(ag_in, 0, [[c.Ta, 128], [1, c.Ta]])
                gp.dma_start(out=ag_in_ap, in_=sb["s_sb"][:, :]).then_inc(
                    sem["g_ag_in"], 16)
                gp.wait_ge(sem["g_ag_in"], 16)
                gp.collective_compute(
                    "AllGather", mybir.AluOpType.bypass, replica_groups=rg,
                    ins=[ag_in[:]], outs=[ag_out[:, :, :]],
                ).then_inc(sem["gc"], 1)
                gp.wait_ge(sem["gc"], GCW + 1)
                # gather back: s_all[p, (k,t)] = ag_out[k, p, t]
                src = bass.AP(ag_out, 0,
                              [[c.Ta, 128], [128 * c.Ta, c.NC], [1, c.Ta]])
                dst = sb["s_all"][:, :].rearrange("p (k t) -> p k t", k=c.NC)
                with nc.allow_non_contiguous_dma(reason="8KB score gather"):
                    gp.dma_start(out=dst, in_=src).then_inc(sem["g_ag_out"], 16)
                # x all-reduce
                gp.wait_ge(sem["dve"], DM["ar1"])
                gp.dma_start(out=ar1_in[:, :], in_=sb["ar1_sb"][:, :]).then_inc(
                    sem["g_ar1_in"], 16)
                gp.wait_ge(sem["g_ar1_in"], 16)
                gp.collective_compute(
                    "AllReduce", mybir.AluOpType.add, replica_groups=rg,
                    ins=[ar1_in[:, :]], outs=[ar1_out[:, :]],
                ).then_inc(sem["gc"], 1)
                gp.wait_ge(sem["gc"], GCW + 2)
                gp.dma_start(out=sb["x_raw"][:, :], in_=ar1_out[:, :]).then_inc(
                    sem["g_ar1_out"], 16)
                # z all-reduce
                gp.wait_ge(sem["dve"], DM["ar2"])
                gp.dma_start(out=ar2_in[:, :], in_=sb["ar2_sb"][:, :]).then_inc(
                    sem["g_ar2_in"], 16)
                gp.wait_ge(sem["g_ar2_in"], 16)
                gp.collective_compute(
                    "AllReduce", mybir.AluOpType.add, replica_groups=rg,
                    ins=[ar2_in[:, :]], outs=[ar2_out[:, :]],
                ).then_inc(sem["gc"], 1)
                gp.wait_ge(sem["gc"], GCW + 3)
                gp.dma_start(out=sb["z_raw"][:, :], in_=ar2_out[:, :]).then_inc(
                    sem["g_ar2_out"], 16)

    # populate .instr bytes for extended-inst ISA subclasses (remote DMA,
    # library reload) — raw Bass skips this pass and walrus then rejects the
    # empty instr with "ISA wrong length"
    mybir.codegen_inst_isa_subclasses(nc)
    return nc


# ---------------------------------------------------------------------------
# entry point
# ---------------------------------------------------------------------------

_NC_CACHE = {}


def _get_nc(c: Cfg):
    key = (c.H, c.L, c.NC, c.RING, c.TPC)
    if key not in _NC_CACHE:
        _NC_CACHE[key] = build_nc(c)
    return _NC_CACHE[key]


def run(inputs: dict, trace: bool = False, tmpdir: str | None = None, cfg: Cfg = CFG):
    from concourse.bass_utils import run_bass_kernel_spmd

    in_maps = prep_inputs(cfg, inputs)
    nc = _get_nc(cfg)
    res = run_bass_kernel_spmd(nc, in_maps, list(range(cfg.NC)), trace=trace,
                               tmpdir=tmpdir)
    outs = decode_outputs(cfg, [r["out"] for r in res.results])
    return outs, res


def kernel(**inputs):
    outs, _ = run(inputs, trace=bool(os.environ.get("KERNEL_TRACE")))
    return outs
